# revision 1
# baseline (speedup 1.0000x reference)
"""ODE-RNN Trainium2 kernel.

Problem: out[b, t*8+i, :] = 2-layer GRU (H=1024) run over the batch dim
(64 steps) of sequence t (30 sequences), with initial hiddens taken from an
RK4-integrated ODE trajectory (8 grid points, shared across all runs).

Strategy (8 NeuronCores, pure data-parallel, no collectives):
  - The ODE trajectory (128 sequential tiny (2,1024) MLP evals, <1.2% of
    FLOPs, latency-serial and weight-streaming-bound on a systolic array)
    is computed on the host in fp32, exactly mirroring the reference math.
  - Core i handles the 30 GRU runs with init traj[i] (data-parallel over the
    240 independent (t,i) runs; weights replicated per core).
  - Per core, the GRU is restructured into 4 phases:
      A: gi1 = X @ wi0.T + bias  (dense, M=2048)             -> DRAM
      B: layer-1 recurrence, 64 steps, state batched M=32    -> h1 states
      C: gi2 = H1states @ wi1.T + bias (dense, M=2048)       -> DRAM
      D: layer-2 recurrence, 64 steps                        -> output
    The recurrent matmuls keep the state transposed ([H,parts] x runs) as the
    PE stationary operand and stream the (resident) recurrent weights as the
    moving operand; the state transpose is maintained with PE transposes.
  - All matmuls run in float32r (1 cycle/row, ~1.5e-4 rel err measured).
"""

import numpy as np

try:
    import concourse.bass as bass  # noqa: F401
except ImportError:  # pragma: no cover - fallback for bare environments
    import sys
    sys.path.insert(0, "/opt/trn_rl_repo")
    import concourse.bass as bass  # noqa: F401

import concourse.mybir as mybir
import concourse.tile as tile
from concourse import bacc
from concourse.bass_utils import run_bass_kernel_spmd
from concourse.masks import make_identity

F32 = mybir.dt.float32
F32R = mybir.dt.float32r
AF = mybir.ActivationFunctionType

H = 1024        # hidden size
G3 = 3 * H      # gate width
KC = H // 128   # K chunks
T = 30          # sequences
R = 32          # padded runs per core (30 real + 2 pad)
NSEG = 8
SUB = 4
NCORES = 8


def build_nc(steps=64):
    """Build the per-core Bass module (same program on all 8 cores)."""
    MT = steps * R            # gi row count (2048 for steps=64)
    MCH = MT // 128           # M chunks (16)
    nc = bacc.Bacc()

    xtr = nc.declare_dram_parameter("xtr", [128, KC, MT], F32R, isOutput=False)
    wi0t = nc.declare_dram_parameter("wi0t", [H, G3], F32R, isOutput=False)
    wh0t = nc.declare_dram_parameter("wh0t", [H, G3], F32R, isOutput=False)
    wi1t = nc.declare_dram_parameter("wi1t", [H, G3], F32R, isOutput=False)
    wh1t = nc.declare_dram_parameter("wh1t", [H, G3], F32R, isOutput=False)
    bias1 = nc.declare_dram_parameter("bias1", [G3], F32, isOutput=False)
    bias2 = nc.declare_dram_parameter("bias2", [G3], F32, isOutput=False)
    bhn1 = nc.declare_dram_parameter("bhn1", [H], F32, isOutput=False)
    bhn2 = nc.declare_dram_parameter("bhn2", [H], F32, isOutput=False)
    h1f0 = nc.declare_dram_parameter("h1f0", [R, H], F32, isOutput=False)
    h2f0 = nc.declare_dram_parameter("h2f0", [R, H], F32, isOutput=False)
    h1t0 = nc.declare_dram_parameter("h1t0", [128, KC, R], F32R, isOutput=False)
    h2t0 = nc.declare_dram_parameter("h2t0", [128, KC, R], F32R, isOutput=False)
    out = nc.declare_dram_parameter("out", [steps, R, H], F32, isOutput=True)

    gi1b = nc.dram_tensor("gi1b", [MT, G3], F32R)
    gi2b = nc.dram_tensor("gi2b", [MT, G3], F32R)
    h1ts = nc.dram_tensor("h1ts", [128, KC, steps, R], F32R)

    def bcast(ap, parts=128):
        return bass.AP(tensor=ap.tensor, offset=ap.offset,
                       ap=[[0, parts]] + list(ap.ap))

    with tile.TileContext(nc) as tc:
        with (
            tc.tile_pool(name="wp", bufs=KC) as wp,
            tc.tile_pool(name="const", bufs=1) as const,
        ):
            # --- constants ---
            bias1_bc = const.tile([128, G3], F32)
            nc.sync.dma_start(out=bias1_bc, in_=bcast(bias1[:]))
            bias2_bc = const.tile([128, G3], F32)
            nc.sync.dma_start(out=bias2_bc, in_=bcast(bias2[:]))
            bhn1_bc = const.tile([R, H], F32)
            nc.sync.dma_start(out=bhn1_bc, in_=bcast(bhn1[:], parts=R))
            bhn2_bc = const.tile([R, H], F32)
            nc.sync.dma_start(out=bhn2_bc, in_=bcast(bhn2[:], parts=R))
            ident_g = const.tile([32, 32], F32)
            make_identity(nc, ident_g)
            ident = const.tile([32, 32], F32)
            nc.vector.tensor_copy(ident, ident_g)
            ident_r = const.tile([32, 32], F32R)
            nc.vector.tensor_copy(ident_r, ident_g)

            # --- state tiles ---
            h1f = const.tile([R, H], F32)
            nc.sync.dma_start(out=h1f, in_=h1f0[:])
            h2f = const.tile([R, H], F32)
            nc.sync.dma_start(out=h2f, in_=h2f0[:])
            h1t = const.tile([128, KC, R], F32R)
            nc.sync.dma_start(out=h1t, in_=h1t0[:])
            h2t = const.tile([128, KC, R], F32R)
            nc.sync.dma_start(out=h2t, in_=h2t0[:])

            def load_weight(param, label):
                tiles = []
                for k in range(KC):
                    wt = wp.tile([128, G3], F32R, tag="w", name=f"w_{label}_{k}")
                    nc.sync.dma_start(out=wt, in_=param[k * 128:(k + 1) * 128, :])
                    tiles.append(wt)
                return tiles

            def phase_gi(wtiles, bias_bc, gib, lhs_loader, nm):
                """gi = lhsT.T @ W + bias for MCH M-chunks of 128 rows."""
                with (
                    tc.tile_pool(name=f"psA{nm}", bufs=2, space="PSUM") as psA,
                    tc.tile_pool(name=f"evp{nm}", bufs=2) as evp,
                    tc.tile_pool(name=f"lhsp{nm}", bufs=2) as lhsp,
                ):
                    lhs = None
                    for m in range(MCH):
                        lhs, msub = lhs_loader(lhsp, m, lhs)
                        for half in range(2):
                            ps = psA.tile([128, 1536], F32, tag="ps", name=f"ps{nm}_{m}_{half}")
                            for n3 in range(3):
                                ncol = half * 1536 + n3 * 512
                                for k in range(KC):
                                    nc.tensor.matmul(
                                        ps[:, n3 * 512:(n3 + 1) * 512],
                                        lhs[:, k, msub * 128:(msub + 1) * 128],
                                        wtiles[k][:, ncol:ncol + 512],
                                        start=(k == 0), stop=(k == KC - 1))
                            ev = evp.tile([128, 1536], F32R, tag="ev", name=f"ev{nm}_{m}_{half}")
                            nc.vector.tensor_add(
                                ev, ps, bias_bc[:, half * 1536:(half + 1) * 1536])
                            nc.sync.dma_start(
                                out=gib[m * 128:(m + 1) * 128,
                                        half * 1536:(half + 1) * 1536],
                                in_=ev)

            def lhs_loader_A(lhsp, m, lhs):
                # xtr chunks: up to 4 M-chunks per DMA ([128, KC, <=512] tiles)
                if m % 4 == 0:
                    width = min(512, (MCH - m) * 128)
                    lhs = lhsp.tile([128, KC, width], F32R, tag="lhsA", name=f"lhsA_{m}")
                    nc.sync.dma_start(
                        out=lhs, in_=xtr[:, :, m * 128:m * 128 + width])
                return lhs, m % 4

            def lhs_loader_C(lhsp, m, lhs):
                # h1ts slice: steps 4m..4m+4 -> [128, KC, 128] (s-major, t-minor)
                lhs = lhsp.tile([128, KC, 4, R], F32R, tag="lhsC", name=f"lhsC_{m}")
                nc.sync.dma_start(out=lhs, in_=h1ts[:, :, 4 * m:4 * m + 4, :])
                return lhs.rearrange("p k s t -> p k (s t)"), 0

            def phase_rec(wtiles, gib, bhn_bc, hf, ht, save, outd, nm):
                with (
                    tc.tile_pool(name=f"ghp{nm}", bufs=7, space="PSUM") as ghp,
                    tc.tile_pool(name=f"trp{nm}", bufs=1, space="PSUM") as trpp,
                    tc.tile_pool(name=f"gp{nm}", bufs=14) as gp,
                    tc.tile_pool(name=f"gip{nm}", bufs=2) as gip,
                ):
                    for s in range(steps):
                        gi = gip.tile([R, G3], F32R, tag="gi", name=f"gi{nm}_{s}")
                        nc.sync.dma_start(out=gi, in_=gib[s * R:(s + 1) * R, :])
                        # K-split accumulation: the k<4 half depends only on
                        # ht chunks 0-3 (rewritten by the previous step's
                        # first gate slice), so it can overlap the previous
                        # step's second-slice gates instead of waiting for
                        # the full state update.
                        ghs = {}
                        for kh in range(2):
                            for n in (0, 2, 4, 1, 3, 5):
                                if kh == 0:
                                    ghs[n] = ghp.tile([R, 512], F32, tag="gh",
                                                      name=f"gh{nm}_{s}_{n}")
                                gh = ghs[n]
                                for k in range(kh * 4, kh * 4 + 4):
                                    nc.tensor.matmul(
                                        gh, ht[:, k, :],
                                        wtiles[k][:, n * 512:(n + 1) * 512],
                                        start=(k == 0),
                                        stop=(k == KC - 1 and n >= 4))
                                if kh == 1 and n < 4:
                                    # r/z gates: accumulate gi (incl. biases)
                                    # on the PE so ACT can sigmoid PSUM
                                    # directly (saves 2 DVE adds per slice).
                                    nc.tensor.matmul(
                                        gh, ident_r, gi[:, n * 512:(n + 1) * 512],
                                        start=False, stop=True)
                        trp = trpp.tile([128, KC, R], F32, tag="tr", name=f"tr{nm}_{s}")
                        for j in range(2):
                            c0 = j * 512
                            t = lambda nmm: gp.tile([R, 512], F32, tag="gt", name=f"{nmm}{nm}_{s}_{j}")
                            rr = t("rr")
                            nc.scalar.activation(rr, ghs[j], AF.Sigmoid)
                            zz = t("zz")
                            nc.scalar.activation(zz, ghs[2 + j], AF.Sigmoid)
                            hn = t("hn")
                            nc.vector.tensor_add(hn, ghs[4 + j], bhn_bc[:, c0:c0 + 512])
                            t1 = t("t1")
                            nc.vector.tensor_mul(t1, rr, hn)
                            npre = t("npre")
                            nc.vector.tensor_add(npre, t1, gi[:, 2 * H + c0:2 * H + c0 + 512])
                            nn = t("nn")
                            nc.scalar.activation(nn, npre, AF.Tanh)
                            dd = t("dd")
                            nc.vector.tensor_sub(dd, hf[:, c0:c0 + 512], nn)
                            t2 = t("t2")
                            nc.vector.tensor_mul(t2, zz, dd)
                            nc.vector.tensor_add(hf[:, c0:c0 + 512], nn, t2)
                            for c in range(j * 4, j * 4 + 4):
                                nc.tensor.transpose(
                                    trp[:, c, :], hf[:, c * 128:(c + 1) * 128], ident)
                            for c in range(j * 4, j * 4 + 4):
                                nc.vector.tensor_copy(ht[:, c, :], trp[:, c, :])
                        if save is not None:
                            nc.sync.dma_start(out=save[:, :, s, :], in_=ht)
                        if outd is not None:
                            nc.sync.dma_start(out=outd[s], in_=hf)

            w = load_weight(wi0t, "i0")
            phase_gi(w, bias1_bc, gi1b, lhs_loader_A, "A")
            w = load_weight(wh0t, "h0")
            phase_rec(w, gi1b, bhn1_bc, h1f, h1t, h1ts, None, "B")
            w = load_weight(wi1t, "i1")
            phase_gi(w, bias2_bc, gi2b, lhs_loader_C, "C")
            w = load_weight(wh1t, "h1")
            phase_rec(w, gi2b, bhn2_bc, h2f, h2t, None, out, "D")

    nc.finalize()
    return nc


def ode_traj(w1, b1, w2, b2, w3, b3):
    """RK4 trajectory of the ODE, mirroring the reference exactly (fp32)."""
    w1t = w1.T.astype(np.float32)
    w2t = w2.T.astype(np.float32)
    w3t = w3.T.astype(np.float32)

    def f(h):
        a = np.tanh(h @ w1t + b1)
        a = np.tanh(a @ w2t + b2)
        return a @ w3t + b3

    dt = np.float32((1.0 / NSEG) / SUB)
    h = np.zeros((2, H), np.float32)
    traj = []
    for _ in range(NSEG):
        for _ in range(SUB):
            k1 = f(h)
            k2 = f(h + np.float32(0.5) * dt * k1)
            k3 = f(h + np.float32(0.5) * dt * k2)
            k4 = f(h + dt * k3)
            h = h + (dt / np.float32(6.0)) * (k1 + np.float32(2.0) * k2
                                              + np.float32(2.0) * k3 + k4)
        traj.append(h.copy())
    return np.stack(traj)  # (NSEG, 2, H)


def make_in_maps(x, w1, b1, w2, b2, w3, b3, wi0, wh0, bi0, bh0,
                 wi1, wh1, bi1, bh1, steps=64, cores=NCORES):
    traj = ode_traj(w1, b1, w2, b2, w3, b3)
    MT = steps * R

    # xtr[p, k, s*R + t] = x[s, t, k*128+p]
    xp = np.zeros((steps, R, H), np.float32)
    xp[:, :T, :] = x[:steps, :, :]
    xtr = np.ascontiguousarray(
        xp.reshape(MT, KC, 128).transpose(2, 1, 0))

    shared = {
        "xtr": xtr,
        "wi0t": np.ascontiguousarray(wi0.T),
        "wh0t": np.ascontiguousarray(wh0.T),
        "wi1t": np.ascontiguousarray(wi1.T),
        "wh1t": np.ascontiguousarray(wh1.T),
        "bias1": np.concatenate([bi0[:2 * H] + bh0[:2 * H], bi0[2 * H:]]),
        "bias2": np.concatenate([bi1[:2 * H] + bh1[:2 * H], bi1[2 * H:]]),
        "bhn1": np.ascontiguousarray(bh0[2 * H:]),
        "bhn2": np.ascontiguousarray(bh1[2 * H:]),
    }
    in_maps = []
    for i in range(cores):
        h1 = traj[i, 0]
        h2 = traj[i, 1]
        m = dict(shared)
        m["h1f0"] = np.ascontiguousarray(np.tile(h1, (R, 1)))
        m["h2f0"] = np.ascontiguousarray(np.tile(h2, (R, 1)))
        m["h1t0"] = np.ascontiguousarray(
            np.broadcast_to(h1.reshape(KC, 128).T[:, :, None], (128, KC, R)))
        m["h2t0"] = np.ascontiguousarray(
            np.broadcast_to(h2.reshape(KC, 128).T[:, :, None], (128, KC, R)))
        in_maps.append(m)
    return in_maps


_NC_CACHE = {}


def _get_nc(steps):
    if steps not in _NC_CACHE:
        _NC_CACHE[steps] = build_nc(steps)
    return _NC_CACHE[steps]


def run_cores(inputs, steps=64, cores=NCORES, **run_kwargs):
    in_maps = make_in_maps(steps=steps, cores=cores, **inputs)
    nc = _get_nc(steps)
    return run_bass_kernel_spmd(nc, in_maps, core_ids=list(range(cores)),
                                **run_kwargs)


def kernel(x, w1, b1, w2, b2, w3, b3, wi0, wh0, bi0, bh0,
           wi1, wh1, bi1, bh1):
    x = np.asarray(x, np.float32)
    args = dict(x=x, w1=w1, b1=b1, w2=w2, b2=b2, w3=w3, b3=b3,
                wi0=wi0, wh0=wh0, bi0=bi0, bh0=bh0,
                wi1=wi1, wh1=wh1, bi1=bi1, bh1=bh1)
    args = {k: np.asarray(v, np.float32) for k, v in args.items()}
    res = run_cores(args, steps=64, cores=NCORES)
    B = 64
    full = np.empty((B, T * NCORES, H), np.float32)
    for i in range(NCORES):
        full[:, i::NCORES, :] = res.results[i]["out"][:, :T, :]
    return full



# revision 9
# speedup vs baseline: 2.6689x; 2.6689x over previous
"""ODE-RNN Trainium2 kernel (v3: transposed-state, weights-stationary bf16).

Problem: out[b, t*8+i, :] = 2-layer GRU (H=1024) run over the batch dim
(64 steps) of sequence t (30 sequences), with initial hiddens taken from an
RK4-integrated ODE trajectory (8 grid points, shared across all runs).

Strategy (8 NeuronCores, pure data-parallel, no collectives):
  - The ODE trajectory (tiny, latency-serial) is computed on the host in
    fp32, exactly mirroring the reference math.
  - Core i handles the 30 GRU runs (padded to R=32) with init traj[i].
  - Everything on-device lives in a TRANSPOSED layout: states/gates are
    [128 H-cols (partitions), chunk, run] so elementwise work uses all 128
    partitions, and recurrence matmuls make the small per-step state the
    MOVING operand (32 bf16 rows/matmul at 1 cycle/row) with the weights
    stationary - ~4x fewer PE row-cycles than streaming the weights.
  - Phase A: gi1 = x @ wi0.T + bias as a dense transposed matmul -> DRAM.
  - Fused step loop: layer-1 gates + update, then layer-2 where the input
    gates wi1.T @ h1_new are accumulated DIRECTLY into the layer-2 gate
    PSUM brackets (no separate gi2 phase, no DRAM roundtrip).
  - PSUM accumulation honors the 2KB zero-region rule: one start/stop
    bracket per bank; the 16 per-chunk results of a bank are disjoint
    32-column slices accumulated inside a single bracket.
  - Gate biases ride on the PE: rank-1 (bias x ones) matmuls; layer-1
    input-gate r/z terms via identity-matmul accumulation of bf16 gi1.
"""

import numpy as np

try:
    import concourse.bass as bass  # noqa: F401
except ImportError:  # pragma: no cover - fallback for bare environments
    import sys
    sys.path.insert(0, "/opt/trn_rl_repo")
    import concourse.bass as bass  # noqa: F401

import ml_dtypes
import concourse.mybir as mybir
import concourse.tile as tile
from concourse import bacc
from concourse.bass_utils import run_bass_kernel_spmd
from concourse.masks import make_identity

F32 = mybir.dt.float32
BF16 = mybir.dt.bfloat16
AF = mybir.ActivationFunctionType
BFNP = ml_dtypes.bfloat16

H = 1024        # hidden size
G3 = 3 * H      # gate width
KC = H // 128   # K chunks (8)
MC = G3 // 128  # gate-col chunks (24)
T = 30          # sequences
R = 32          # padded runs per core (30 real + 2 pad)
NSEG = 8
SUB = 4
NCORES = 8

# brow layout offsets (bf16 row vector of per-gate-col biases)
OFF_BHN1 = 0          # bh0 n-part      (H)
OFF_B2RZ = H          # (bi1+bh1) r/z   (2H)
OFF_BHN2 = 3 * H      # bh1 n-part      (H)
OFF_B2N = 4 * H       # bi1 n-part      (H)


def _rz_off(m):
    """Free offset of gate-chunk m (0..15 = r0-7, z0-7) inside the rz bank."""
    return m * 32 if m < 8 else 256 + (m - 8) * 32


def build_nc(steps=64):
    """Build the per-core Bass module (same program on all 8 cores)."""
    MT = steps * R            # gi1 row count
    nc = bacc.Bacc()

    xtr = nc.declare_dram_parameter("xtr", [128, KC, MT], BF16, isOutput=False)
    wi0s = nc.declare_dram_parameter("wi0s", [128, KC, G3], BF16, isOutput=False)
    wh0s = nc.declare_dram_parameter("wh0s", [128, KC, G3], BF16, isOutput=False)
    wi1s = nc.declare_dram_parameter("wi1s", [128, KC, G3], BF16, isOutput=False)
    wh1s = nc.declare_dram_parameter("wh1s", [128, KC, G3], BF16, isOutput=False)
    bias1c = nc.declare_dram_parameter("bias1c", [128, MC], F32, isOutput=False)
    brow = nc.declare_dram_parameter("brow", [1, 5 * H], BF16, isOutput=False)
    h1t0 = nc.declare_dram_parameter("h1t0", [128, KC, R], F32, isOutput=False)
    h2t0 = nc.declare_dram_parameter("h2t0", [128, KC, R], F32, isOutput=False)
    out = nc.declare_dram_parameter("out", [steps, 128, KC, R], F32,
                                    isOutput=True)

    gi1b = nc.dram_tensor("gi1b", [128, MC, MT], BF16)

    with tile.TileContext(nc) as tc:
        with (
            tc.tile_pool(name="wp", bufs=3 * KC) as wp,
            tc.tile_pool(name="const", bufs=1) as const,
        ):
            # --- constants ---
            bias1c_t = const.tile([128, MC], F32)
            nc.sync.dma_start(out=bias1c_t, in_=bias1c[:])
            brow_t = const.tile([1, 5 * H], BF16)
            nc.sync.dma_start(out=brow_t, in_=brow[:])
            ident_f = const.tile([128, 128], F32)
            make_identity(nc, ident_f)
            ident = const.tile([128, 128], BF16)
            nc.vector.tensor_copy(ident, ident_f)
            ones = const.tile([1, R], BF16)
            nc.vector.memset(ones, 1.0)

            def load_weight(param, label):
                tiles = []
                for k in range(KC):
                    wt = wp.tile([128, G3], BF16, tag="w", name=f"w_{label}_{k}")
                    nc.sync.dma_start(out=wt, in_=param[:, k, :])
                    tiles.append(wt)
                return tiles

            wA = load_weight(wi0s, "i0")

            # ---------------- Phase A: gi1 (transposed dense) -------------
            NRB = MT // 512   # 512-row blocks
            with (
                tc.tile_pool(name="psA", bufs=8, space="PSUM") as psA,
                tc.tile_pool(name="evA", bufs=6) as evA,
                tc.tile_pool(name="xp", bufs=1) as xp,
            ):
                xt = xp.tile([128, KC, MT], BF16)
                nc.sync.dma_start(out=xt, in_=xtr[:])
                for m in range(MC):
                    pss = [psA.tile([128, 512], F32, tag="psA",
                                    name=f"psA_{m}_{rb}") for rb in range(NRB)]
                    for k in range(KC):
                        for rb in range(NRB):
                            nc.tensor.matmul(
                                pss[rb], wA[k][:, m * 128:(m + 1) * 128],
                                xt[:, k, rb * 512:(rb + 1) * 512],
                                start=(k == 0), stop=(k == KC - 1))
                    for rb in range(NRB):
                        ev = evA.tile([128, 512], BF16, tag="ev",
                                      name=f"ev_{m}_{rb}")
                        nc.scalar.activation(ev, pss[rb], AF.Identity,
                                             bias=bias1c_t[:, m:m + 1])
                        nc.sync.dma_start(
                            out=gi1b[:, m, rb * 512:(rb + 1) * 512], in_=ev)

            # weights for the fused loop (wh1 recycles wi0's slots)
            wB = load_weight(wh0s, "h0")
            wC = load_weight(wi1s, "i1")
            wD = load_weight(wh1s, "h1")

            # ---------------- Fused recurrence loop -----------------------
            with (
                tc.tile_pool(name="psB", bufs=2, space="PSUM") as psBp,
                tc.tile_pool(name="psD", bufs=1, space="PSUM") as psDp,
                tc.tile_pool(name="hs1p", bufs=2) as hs1p,
                tc.tile_pool(name="hs2p", bufs=2) as hs2p,
                tc.tile_pool(name="htb1p", bufs=2) as htb1p,
                tc.tile_pool(name="htb2p", bufs=2) as htb2p,
                tc.tile_pool(name="gip", bufs=4) as gip,
                tc.tile_pool(name="gp", bufs=3) as gp,
            ):
                hs1 = hs1p.tile([128, KC, R], F32, tag="hs1", name="hs1_init")
                nc.sync.dma_start(out=hs1, in_=h1t0[:])
                hs2 = hs2p.tile([128, KC, R], F32, tag="hs2", name="hs2_init")
                nc.sync.dma_start(out=hs2, in_=h2t0[:])
                htb1 = htb1p.tile([128, KC, R], BF16, tag="htb1", name="htb1_init")
                nc.scalar.activation(htb1, hs1, AF.Copy)
                htb2 = htb2p.tile([128, KC, R], BF16, tag="htb2", name="htb2_init")
                nc.scalar.activation(htb2, hs2, AF.Copy)

                def rank1(dst, boff, m, stop):
                    nc.tensor.matmul(
                        dst, brow_t[:, boff + m * 128:boff + (m + 1) * 128],
                        ones, start=False, stop=stop)

                def update(nm, s, ps, gin_ap, hs_cur, hs_new, htb_new):
                    t = lambda lbl: gp.tile([128, KC, R], F32, tag=lbl,
                                            name=f"{lbl}{nm}_{s}")
                    rs = t("rs")
                    nc.scalar.activation(rs, ps[:, 0, 0:256], AF.Sigmoid)
                    zs = t("zs")
                    nc.scalar.activation(zs, ps[:, 0, 256:512], AF.Sigmoid)
                    t1 = t("t1")
                    nc.vector.tensor_mul(t1, rs, ps[:, 1, 0:256])
                    npre = t("np")
                    nc.vector.tensor_add(npre, t1, gin_ap)
                    nn = t("nn")
                    nc.scalar.activation(nn, npre, AF.Tanh)
                    dd = t("dd")
                    nc.vector.tensor_sub(dd, hs_cur, nn)
                    t2 = t("t2")
                    nc.vector.tensor_mul(t2, zs, dd)
                    nc.vector.tensor_add(hs_new, nn, t2)
                    nc.scalar.activation(htb_new, hs_new, AF.Copy)

                for s in range(steps):
                    gi = gip.tile([128, MC, R], BF16, tag="gi", name=f"gi_{s}")
                    nc.sync.dma_start(out=gi, in_=gi1b[:, :, s * R:(s + 1) * R])

                    # --- layer-1 r/z bank: gh1 + gi1 (one bracket) ---
                    psb = psBp.tile([128, 2, 512], F32, tag="psb",
                                    name=f"psb_{s}")
                    for m in range(16):
                        dst = psb[:, 0, _rz_off(m):_rz_off(m) + 32]
                        for k in range(KC):
                            nc.tensor.matmul(
                                dst, wB[k][:, m * 128:(m + 1) * 128],
                                htb1[:, k, :],
                                start=(m == 0 and k == 0), stop=False)
                        nc.tensor.matmul(dst, ident, gi[:, m, :],
                                         start=False, stop=(m == 15))
                    # --- layer-1 n bank: gh1_n + bhn1 ---
                    for j in range(8):
                        dst = psb[:, 1, j * 32:(j + 1) * 32]
                        for k in range(KC):
                            nc.tensor.matmul(
                                dst, wB[k][:, (16 + j) * 128:(17 + j) * 128],
                                htb1[:, k, :],
                                start=(j == 0 and k == 0), stop=False)
                        rank1(dst, OFF_BHN1, j, stop=(j == 7))

                    # --- layer-2: gh2 parts (need only htb2) ---
                    psd = psDp.tile([128, 3, 512], F32, tag="psd",
                                    name=f"psd_{s}")
                    for m in range(16):
                        dst = psd[:, 0, _rz_off(m):_rz_off(m) + 32]
                        for k in range(KC):
                            nc.tensor.matmul(
                                dst, wD[k][:, m * 128:(m + 1) * 128],
                                htb2[:, k, :],
                                start=(m == 0 and k == 0), stop=False)
                    for j in range(8):
                        dst = psd[:, 1, j * 32:(j + 1) * 32]
                        for k in range(KC):
                            nc.tensor.matmul(
                                dst, wD[k][:, (16 + j) * 128:(17 + j) * 128],
                                htb2[:, k, :],
                                start=(j == 0 and k == 0), stop=False)
                        rank1(dst, OFF_BHN2, j, stop=(j == 7))

                    # --- layer-1 update chain (ACT/DVE) ---
                    hs1n = hs1p.tile([128, KC, R], F32, tag="hs1",
                                     name=f"hs1_{s}")
                    htb1n = htb1p.tile([128, KC, R], BF16, tag="htb1",
                                       name=f"htb1_{s}")
                    update("B", s, psb, gi[:, 16:24, :], hs1, hs1n, htb1n)

                    # --- layer-2: gi2 parts (need htb1n) + biases ---
                    for m in range(16):
                        dst = psd[:, 0, _rz_off(m):_rz_off(m) + 32]
                        for k in range(KC):
                            nc.tensor.matmul(
                                dst, wC[k][:, m * 128:(m + 1) * 128],
                                htb1n[:, k, :], start=False, stop=False)
                        rank1(dst, OFF_B2RZ, m, stop=(m == 15))
                    for j in range(8):
                        dst = psd[:, 2, j * 32:(j + 1) * 32]
                        for k in range(KC):
                            nc.tensor.matmul(
                                dst, wC[k][:, (16 + j) * 128:(17 + j) * 128],
                                htb1n[:, k, :],
                                start=(j == 0 and k == 0), stop=False)
                        rank1(dst, OFF_B2N, j, stop=(j == 7))

                    # --- layer-2 update chain + output ---
                    hs2n = hs2p.tile([128, KC, R], F32, tag="hs2",
                                     name=f"hs2_{s}")
                    htb2n = htb2p.tile([128, KC, R], BF16, tag="htb2",
                                       name=f"htb2_{s}")
                    update("D", s, psd, psd[:, 2, 0:256], hs2, hs2n, htb2n)
                    nc.sync.dma_start(out=out[s], in_=hs2n)

                    hs1, htb1, hs2, htb2 = hs1n, htb1n, hs2n, htb2n

    nc.finalize()
    return nc


def ode_traj(w1, b1, w2, b2, w3, b3):
    """RK4 trajectory of the ODE, mirroring the reference exactly (fp32)."""
    w1t = w1.T.astype(np.float32)
    w2t = w2.T.astype(np.float32)
    w3t = w3.T.astype(np.float32)

    def f(h):
        a = np.tanh(h @ w1t + b1)
        a = np.tanh(a @ w2t + b2)
        return a @ w3t + b3

    dt = np.float32((1.0 / NSEG) / SUB)
    h = np.zeros((2, H), np.float32)
    traj = []
    for _ in range(NSEG):
        for _ in range(SUB):
            k1 = f(h)
            k2 = f(h + np.float32(0.5) * dt * k1)
            k3 = f(h + np.float32(0.5) * dt * k2)
            k4 = f(h + dt * k3)
            h = h + (dt / np.float32(6.0)) * (k1 + np.float32(2.0) * k2
                                              + np.float32(2.0) * k3 + k4)
        traj.append(h.copy())
    return np.stack(traj)  # (NSEG, 2, H)


def _wstat(w):
    """[G3, H] weight -> stationary layout [128, KC, G3] (bf16)."""
    return np.ascontiguousarray(
        w.T.reshape(KC, 128, G3).transpose(1, 0, 2)).astype(BFNP)


def make_in_maps(x, w1, b1, w2, b2, w3, b3, wi0, wh0, bi0, bh0,
                 wi1, wh1, bi1, bh1, steps=64, cores=NCORES):
    traj = ode_traj(w1, b1, w2, b2, w3, b3)
    MT = steps * R

    xp = np.zeros((steps, R, H), np.float32)
    xp[:, :T, :] = x[:steps, :, :]
    xtr = np.ascontiguousarray(
        xp.reshape(MT, KC, 128).transpose(2, 1, 0)).astype(BFNP)

    bias1full = np.concatenate([bi0[:2 * H] + bh0[:2 * H], bi0[2 * H:]])
    brow = np.concatenate([bh0[2 * H:],                      # OFF_BHN1
                           bi1[:2 * H] + bh1[:2 * H],        # OFF_B2RZ
                           bh1[2 * H:],                      # OFF_BHN2
                           bi1[2 * H:]])                     # OFF_B2N

    shared = {
        "xtr": xtr,
        "wi0s": _wstat(wi0), "wh0s": _wstat(wh0),
        "wi1s": _wstat(wi1), "wh1s": _wstat(wh1),
        "bias1c": np.ascontiguousarray(
            bias1full.reshape(MC, 128).T).astype(np.float32),
        "brow": brow[None].astype(BFNP),
    }
    in_maps = []
    for i in range(cores):
        m = dict(shared)
        for nm, hv in (("h1t0", traj[i, 0]), ("h2t0", traj[i, 1])):
            ht = np.broadcast_to(
                hv.reshape(KC, 128).T[:, :, None], (128, KC, R))
            m[nm] = np.ascontiguousarray(ht).astype(np.float32)
        in_maps.append(m)
    return in_maps


_NC_CACHE = {}


def _get_nc(steps):
    if steps not in _NC_CACHE:
        _NC_CACHE[steps] = build_nc(steps)
    return _NC_CACHE[steps]


def run_cores(inputs, steps=64, cores=NCORES, **run_kwargs):
    in_maps = make_in_maps(steps=steps, cores=cores, **inputs)
    nc = _get_nc(steps)
    return run_bass_kernel_spmd(nc, in_maps, core_ids=list(range(cores)),
                                **run_kwargs)


def kernel(x, w1, b1, w2, b2, w3, b3, wi0, wh0, bi0, bh0,
           wi1, wh1, bi1, bh1):
    args = dict(x=x, w1=w1, b1=b1, w2=w2, b2=b2, w3=w3, b3=b3,
                wi0=wi0, wh0=wh0, bi0=bi0, bh0=bh0,
                wi1=wi1, wh1=wh1, bi1=bi1, bh1=bh1)
    args = {k: np.asarray(v, np.float32) for k, v in args.items()}
    res = run_cores(args, steps=64, cores=NCORES)
    B = 64
    full = np.empty((B, T * NCORES, H), np.float32)
    for i in range(NCORES):
        r = np.asarray(res.results[i]["out"], np.float32)  # (64,128,KC,R)
        r = np.transpose(r, (0, 3, 2, 1)).reshape(B, R, H)
        full[:, i::NCORES, :] = r[:, :T, :]
    return full


# revision 15
# speedup vs baseline: 2.9856x; 1.1187x over previous
"""ODE-RNN Trainium2 kernel (v3: transposed-state, weights-stationary bf16).

Problem: out[b, t*8+i, :] = 2-layer GRU (H=1024) run over the batch dim
(64 steps) of sequence t (30 sequences), with initial hiddens taken from an
RK4-integrated ODE trajectory (8 grid points, shared across all runs).

Strategy (8 NeuronCores, pure data-parallel, no collectives):
  - The ODE trajectory (tiny, latency-serial) is computed on the host in
    fp32, exactly mirroring the reference math.
  - Core i handles the 30 GRU runs (padded to R=32) with init traj[i].
  - Everything on-device lives in a TRANSPOSED layout: states/gates are
    [128 H-cols (partitions), chunk, run] so elementwise work uses all 128
    partitions, and recurrence matmuls make the small per-step state the
    MOVING operand (32 bf16 rows/matmul at 1 cycle/row) with the weights
    stationary - ~4x fewer PE row-cycles than streaming the weights.
  - Phase A: gi1 = x @ wi0.T + bias as a dense transposed matmul -> DRAM.
  - Fused step loop: layer-1 gates + update, then layer-2 where the input
    gates wi1.T @ h1_new are accumulated DIRECTLY into the layer-2 gate
    PSUM brackets (no separate gi2 phase, no DRAM roundtrip).
  - PSUM accumulation honors the 2KB zero-region rule: one start/stop
    bracket per bank; the 16 per-chunk results of a bank are disjoint
    32-column slices accumulated inside a single bracket.
  - Gate biases ride on the PE: rank-1 (bias x ones) matmuls; layer-1
    input-gate r/z terms via identity-matmul accumulation of bf16 gi1.
"""

import numpy as np

try:
    import concourse.bass as bass  # noqa: F401
except ImportError:  # pragma: no cover - fallback for bare environments
    import sys
    sys.path.insert(0, "/opt/trn_rl_repo")
    import concourse.bass as bass  # noqa: F401

import ml_dtypes
import concourse.mybir as mybir
import concourse.tile as tile
from concourse import bacc
from concourse.bass_utils import run_bass_kernel_spmd
from concourse.masks import make_identity

F32 = mybir.dt.float32
BF16 = mybir.dt.bfloat16
AF = mybir.ActivationFunctionType
BFNP = ml_dtypes.bfloat16

H = 1024        # hidden size
G3 = 3 * H      # gate width
KC = H // 128   # K chunks (8)
MC = G3 // 128  # gate-col chunks (24)
T = 30          # sequences
R = 32          # padded runs per core (30 real + 2 pad)
NSEG = 8
SUB = 4
NCORES = 8

# brow layout offsets (bf16 row vector of per-gate-col biases)
OFF_BHN1 = 0          # bh0 n-part      (H)
OFF_B2RZ = H          # (bi1+bh1) r/z   (2H)
OFF_BHN2 = 3 * H      # bh1 n-part      (H)
OFF_B2N = 4 * H       # bi1 n-part      (H)


def _rz_off(m):
    """Free offset of gate-chunk m (0..15 = r0-7, z0-7) inside the rz bank."""
    return m * 32 if m < 8 else 256 + (m - 8) * 32


def build_nc(steps=64):
    """Build the per-core Bass module (same program on all 8 cores)."""
    MT = steps * R            # gi1 row count
    nc = bacc.Bacc()

    xtr = nc.declare_dram_parameter("xtr", [128, KC, MT], BF16, isOutput=False)
    wi0s = nc.declare_dram_parameter("wi0s", [128, KC, G3], BF16, isOutput=False)
    wh0s = nc.declare_dram_parameter("wh0s", [128, KC, G3], BF16, isOutput=False)
    wi1s = nc.declare_dram_parameter("wi1s", [128, KC, G3], BF16, isOutput=False)
    wh1s = nc.declare_dram_parameter("wh1s", [128, KC, G3], BF16, isOutput=False)
    bias1c = nc.declare_dram_parameter("bias1c", [128, MC], F32, isOutput=False)
    brow = nc.declare_dram_parameter("brow", [1, 5 * H], BF16, isOutput=False)
    h1t0 = nc.declare_dram_parameter("h1t0", [128, KC, R], BF16, isOutput=False)
    h2t0 = nc.declare_dram_parameter("h2t0", [128, KC, R], BF16, isOutput=False)
    out = nc.declare_dram_parameter("out", [steps, 128, KC, R], BF16,
                                    isOutput=True)

    gi1b = nc.dram_tensor("gi1b", [128, MC, MT], BF16)

    with tile.TileContext(nc) as tc:
        with (
            tc.tile_pool(name="wp", bufs=3 * KC) as wp,
            tc.tile_pool(name="const", bufs=1) as const,
        ):
            # --- constants ---
            bias1c_t = const.tile([128, MC], F32)
            nc.sync.dma_start(out=bias1c_t, in_=bias1c[:])
            brow_t = const.tile([1, 5 * H], BF16)
            nc.sync.dma_start(out=brow_t, in_=brow[:])
            ident_f = const.tile([128, 128], F32)
            make_identity(nc, ident_f)
            ident = const.tile([128, 128], BF16)
            nc.vector.tensor_copy(ident, ident_f)
            ones = const.tile([1, R], BF16)
            nc.vector.memset(ones, 1.0)

            def load_weight(param, label):
                tiles = []
                for k in range(KC):
                    wt = wp.tile([128, G3], BF16, tag="w", name=f"w_{label}_{k}")
                    nc.sync.dma_start(out=wt, in_=param[:, k, :])
                    tiles.append(wt)
                return tiles

            wA = load_weight(wi0s, "i0")

            # ---------------- Phase A: gi1 (transposed dense) -------------
            NRB = MT // 512   # 512-row blocks
            with (
                tc.tile_pool(name="psA", bufs=8, space="PSUM") as psA,
                tc.tile_pool(name="evA", bufs=6) as evA,
                tc.tile_pool(name="xp", bufs=1) as xp,
            ):
                xt = xp.tile([128, KC, MT], BF16)
                nc.sync.dma_start(out=xt, in_=xtr[:])
                for m in range(MC):
                    pss = [psA.tile([128, 512], F32, tag="psA",
                                    name=f"psA_{m}_{rb}") for rb in range(NRB)]
                    for k in range(KC):
                        for rb in range(NRB):
                            nc.tensor.matmul(
                                pss[rb], wA[k][:, m * 128:(m + 1) * 128],
                                xt[:, k, rb * 512:(rb + 1) * 512],
                                start=(k == 0), stop=(k == KC - 1))
                    for rb in range(NRB):
                        ev = evA.tile([128, 512], BF16, tag="ev",
                                      name=f"ev_{m}_{rb}")
                        nc.scalar.activation(ev, pss[rb], AF.Identity,
                                             bias=bias1c_t[:, m:m + 1])
                        nc.sync.dma_start(
                            out=gi1b[:, m, rb * 512:(rb + 1) * 512], in_=ev)

            # weights for the fused loop (wh1 recycles wi0's slots)
            wB = load_weight(wh0s, "h0")
            wC = load_weight(wi1s, "i1")
            wD = load_weight(wh1s, "h1")

            # ---------------- Fused recurrence loop -----------------------
            # PSUM bank layout, one 2KB bank-bracket per (layer, H-half):
            #   psB bank h: [r-h (4x32) | z-h | n-h | unused]
            #   psD bank h: [r-h | z-h | ngh-h | ngi-h]
            # Half h covers H-cols [512h, 512h+512) = state chunks 4h..4h+3,
            # i.e. gate m-chunks r: 4h+c, z: 8+4h+c, n: 16+4h+c (c in 0..3).
            # The loop is software-pipelined: iteration s issues the layer-1
            # bracket+chain for step s+1 BEFORE layer-2 of step s, so the PE
            # never waits on a chain issued in the same iteration.
            with (
                tc.tile_pool(name="psB0", bufs=2, space="PSUM") as psB0p,
                tc.tile_pool(name="psB1", bufs=2, space="PSUM") as psB1p,
                tc.tile_pool(name="psD0", bufs=2, space="PSUM") as psD0p,
                tc.tile_pool(name="psD1", bufs=2, space="PSUM") as psD1p,
                tc.tile_pool(name="htb1p", bufs=2) as htb1p,
                tc.tile_pool(name="htb2p", bufs=2) as htb2p,
                tc.tile_pool(name="gip", bufs=4) as gip,
                tc.tile_pool(name="gp", bufs=5) as gp,
            ):
                htb1 = htb1p.tile([128, KC, R], BF16, tag="htb1", name="htb1_init")
                nc.sync.dma_start(out=htb1, in_=h1t0[:])
                htb2 = htb2p.tile([128, KC, R], BF16, tag="htb2", name="htb2_init")
                nc.sync.dma_start(out=htb2, in_=h2t0[:])

                def rank1(dst, boff, m, stop):
                    nc.tensor.matmul(
                        dst, brow_t[:, boff + m * 128:boff + (m + 1) * 128],
                        ones, start=False, stop=stop)

                def gh_group(ps, h, q, c, w, htb, start):
                    """8 recurrent matmuls into quarter q, slot c of bank h."""
                    m = (0, 8, 16, 16)[q] + 4 * h + c
                    dst = ps[h][:, q * 128 + c * 32:q * 128 + (c + 1) * 32]
                    for k in range(KC):
                        nc.tensor.matmul(
                            dst, w[k][:, m * 128:(m + 1) * 128], htb[:, k, :],
                            start=(start and k == 0), stop=False)

                def update_half(nm, s, h, ps, gin_ap, htb_cur, htb_new):
                    a, b = 4 * h, 4 * h + 4
                    t = lambda lbl, sh: gp.tile([128, sh, R], F32, tag=lbl,
                                                name=f"{lbl}{nm}_{s}_{h}")
                    rz = t("rz", 8)
                    nc.scalar.activation(rz, ps[h][:, 0:256], AF.Sigmoid)
                    t1 = t("t1", 4)
                    nc.vector.tensor_mul(t1, rz[:, 0:4, :], ps[h][:, 256:384])
                    npre = t("np", 4)
                    nc.vector.tensor_add(npre, t1, gin_ap)
                    nn = t("nn", 4)
                    nc.scalar.activation(nn, npre, AF.Tanh)
                    dd = t("dd", 4)
                    nc.vector.tensor_sub(dd, htb_cur[:, a:b, :], nn)
                    t2 = t("t2", 4)
                    nc.vector.tensor_mul(t2, rz[:, 4:8, :], dd)
                    nc.vector.tensor_add(htb_new[:, a:b, :], nn, t2)

                def bracket_B(s, psb, gi, htb_cur):
                    for h in (0, 1):
                        for q in range(3):
                            for c in range(4):
                                gh_group(psb, h, q, c, wB, htb_cur,
                                         start=(q == 0 and c == 0))
                                m = (0, 8, 16)[q] + 4 * h + c
                                dst = psb[h][:, q * 128 + c * 32:
                                             q * 128 + (c + 1) * 32]
                                if q < 2:
                                    nc.tensor.matmul(
                                        dst, ident, gi[:, m, :], start=False,
                                        stop=(q == 2 and c == 3))
                                else:
                                    rank1(dst, OFF_BHN1, 4 * h + c,
                                          stop=(q == 2 and c == 3))

                # software-pipelined steady state; gi prefetched 2 steps out
                def gi_fetch(j):
                    g = gip.tile([128, MC, R], BF16, tag="gi", name=f"gi_{j}")
                    nc.sync.dma_start(out=g, in_=gi1b[:, :, j * R:(j + 1) * R])
                    return g

                gi_tiles = {j: gi_fetch(j) for j in range(min(3, steps))}
                for s in range(-1, steps):
                    if s + 4 < steps:
                        gi_tiles[s + 4] = gi_fetch(s + 4)
                    # --- issue layer-1 of step s+1 ---
                    if s + 1 < steps:
                        gi_n = gi_tiles.pop(s + 1)
                        psb_n = [
                            psB0p.tile([128, 512], F32, tag="psb0",
                                       name=f"psb0_{s + 1}"),
                            psB1p.tile([128, 512], F32, tag="psb1",
                                       name=f"psb1_{s + 1}")]
                        bracket_B(s + 1, psb_n, gi_n, htb1)
                        htb1n = htb1p.tile([128, KC, R], BF16, tag="htb1",
                                           name=f"htb1_{s + 1}")
                        update_half("B", s + 1, 0, psb_n, gi_n[:, 16:20, :],
                                    htb1, htb1n)
                        update_half("B", s + 1, 1, psb_n, gi_n[:, 20:24, :],
                                    htb1, htb1n)
                    else:
                        htb1n = None

                    if s >= 0:
                        # --- layer-2 of step s (htb1 = state after step s) ---
                        psd = [
                            psD0p.tile([128, 512], F32, tag="psd0",
                                       name=f"psd0_{s}"),
                            psD1p.tile([128, 512], F32, tag="psd1",
                                       name=f"psd1_{s}")]
                        for h in (0, 1):
                            for q in range(3):
                                for c in range(4):
                                    gh_group(psd, h, q, c, wD, htb2,
                                             start=(q == 0 and c == 0))
                                    if q == 2:
                                        dst = psd[h][:, 256 + c * 32:
                                                     256 + (c + 1) * 32]
                                        rank1(dst, OFF_BHN2, 4 * h + c,
                                              stop=False)
                        for h in (0, 1):
                            for q in range(4):
                                qq = (0, 8, 0, 16)[q]
                                for c in range(4):
                                    if q == 2:
                                        continue
                                    m = qq + 4 * h + c
                                    dst = psd[h][:, q * 128 + c * 32:
                                                 q * 128 + (c + 1) * 32]
                                    for k in range(KC):
                                        nc.tensor.matmul(
                                            dst,
                                            wC[k][:, m * 128:(m + 1) * 128],
                                            htb1[:, k, :],
                                            start=False, stop=False)
                            for c in range(4):
                                rank1(psd[h][:, c * 32:(c + 1) * 32],
                                      OFF_B2RZ, 4 * h + c, stop=False)
                                rank1(psd[h][:, 128 + c * 32:128 + (c + 1) * 32],
                                      OFF_B2RZ, 8 + 4 * h + c, stop=False)
                                rank1(psd[h][:, 384 + c * 32:384 + (c + 1) * 32],
                                      OFF_B2N, 4 * h + c, stop=(c == 3))

                        htb2n = htb2p.tile([128, KC, R], BF16, tag="htb2",
                                           name=f"htb2_{s}")
                        update_half("D", s, 0, psd, psd[0][:, 384:512],
                                    htb2, htb2n)
                        update_half("D", s, 1, psd, psd[1][:, 384:512],
                                    htb2, htb2n)
                        nc.sync.dma_start(out=out[s], in_=htb2n)
                        htb2 = htb2n

                    if htb1n is not None:
                        htb1 = htb1n

    nc.finalize()
    return nc


def ode_traj(w1, b1, w2, b2, w3, b3):
    """RK4 trajectory of the ODE, mirroring the reference exactly (fp32)."""
    w1t = w1.T.astype(np.float32)
    w2t = w2.T.astype(np.float32)
    w3t = w3.T.astype(np.float32)

    def f(h):
        a = np.tanh(h @ w1t + b1)
        a = np.tanh(a @ w2t + b2)
        return a @ w3t + b3

    dt = np.float32((1.0 / NSEG) / SUB)
    h = np.zeros((2, H), np.float32)
    traj = []
    for _ in range(NSEG):
        for _ in range(SUB):
            k1 = f(h)
            k2 = f(h + np.float32(0.5) * dt * k1)
            k3 = f(h + np.float32(0.5) * dt * k2)
            k4 = f(h + dt * k3)
            h = h + (dt / np.float32(6.0)) * (k1 + np.float32(2.0) * k2
                                              + np.float32(2.0) * k3 + k4)
        traj.append(h.copy())
    return np.stack(traj)  # (NSEG, 2, H)


def _wstat(w):
    """[G3, H] weight -> stationary layout [128, KC, G3] (bf16)."""
    return np.ascontiguousarray(
        w.T.reshape(KC, 128, G3).transpose(1, 0, 2)).astype(BFNP)


def make_in_maps(x, w1, b1, w2, b2, w3, b3, wi0, wh0, bi0, bh0,
                 wi1, wh1, bi1, bh1, steps=64, cores=NCORES):
    traj = ode_traj(w1, b1, w2, b2, w3, b3)
    MT = steps * R

    xp = np.zeros((steps, R, H), np.float32)
    xp[:, :T, :] = x[:steps, :, :]
    xtr = np.ascontiguousarray(
        xp.reshape(MT, KC, 128).transpose(2, 1, 0)).astype(BFNP)

    bias1full = np.concatenate([bi0[:2 * H] + bh0[:2 * H], bi0[2 * H:]])
    brow = np.concatenate([bh0[2 * H:],                      # OFF_BHN1
                           bi1[:2 * H] + bh1[:2 * H],        # OFF_B2RZ
                           bh1[2 * H:],                      # OFF_BHN2
                           bi1[2 * H:]])                     # OFF_B2N

    shared = {
        "xtr": xtr,
        "wi0s": _wstat(wi0), "wh0s": _wstat(wh0),
        "wi1s": _wstat(wi1), "wh1s": _wstat(wh1),
        "bias1c": np.ascontiguousarray(
            bias1full.reshape(MC, 128).T).astype(np.float32),
        "brow": brow[None].astype(BFNP),
    }
    in_maps = []
    for i in range(cores):
        m = dict(shared)
        for nm, hv in (("h1t0", traj[i, 0]), ("h2t0", traj[i, 1])):
            ht = np.broadcast_to(
                hv.reshape(KC, 128).T[:, :, None], (128, KC, R))
            m[nm] = np.ascontiguousarray(ht).astype(BFNP)
        in_maps.append(m)
    return in_maps


_NC_CACHE = {}


def _get_nc(steps):
    if steps not in _NC_CACHE:
        _NC_CACHE[steps] = build_nc(steps)
    return _NC_CACHE[steps]


def run_cores(inputs, steps=64, cores=NCORES, **run_kwargs):
    in_maps = make_in_maps(steps=steps, cores=cores, **inputs)
    nc = _get_nc(steps)
    return run_bass_kernel_spmd(nc, in_maps, core_ids=list(range(cores)),
                                **run_kwargs)


def kernel(x, w1, b1, w2, b2, w3, b3, wi0, wh0, bi0, bh0,
           wi1, wh1, bi1, bh1):
    args = dict(x=x, w1=w1, b1=b1, w2=w2, b2=b2, w3=w3, b3=b3,
                wi0=wi0, wh0=wh0, bi0=bi0, bh0=bh0,
                wi1=wi1, wh1=wh1, bi1=bi1, bh1=bh1)
    args = {k: np.asarray(v, np.float32) for k, v in args.items()}
    res = run_cores(args, steps=64, cores=NCORES)
    B = 64
    full = np.empty((B, T * NCORES, H), np.float32)
    for i in range(NCORES):
        r = np.asarray(res.results[i]["out"], np.float32)  # (64,128,KC,R)
        r = np.transpose(r, (0, 3, 2, 1)).reshape(B, R, H)
        full[:, i::NCORES, :] = r[:, :T, :]
    return full


# revision 23
# speedup vs baseline: 3.1571x; 1.0574x over previous
"""ODE-RNN Trainium2 kernel (transposed-state, weights-stationary bf16).

Problem: out[b, t*8+i, :] = 2-layer GRU (H=1024) run over the batch dim
(64 steps) of sequence t (30 sequences), with initial hiddens taken from an
RK4-integrated ODE trajectory (8 grid points, shared across all runs).

Strategy (8 NeuronCores, pure data-parallel, no collectives):
  - The ODE trajectory (tiny, latency-serial) is computed on the host in
    fp32, exactly mirroring the reference math.
  - Core i handles the 30 GRU runs with init traj[i] (data-parallel over
    the 240 independent (t,i) runs; weights replicated per core).
  - Everything on-device lives in a TRANSPOSED layout: states/gates are
    [128 H-cols (partitions), chunk, run] so elementwise work uses all 128
    partitions, and recurrence matmuls make the small per-step state the
    MOVING operand (30 bf16 rows/matmul at 1 cycle/row) with the weights
    stationary - ~4x fewer PE row-cycles than streaming the weights.
  - Phase A: gi1 = x @ wi0.T + bias as a dense transposed matmul -> DRAM
    (bias folded into the PSUM->bf16 eviction on the Activation engine).
  - Software-pipelined fused step loop: iteration s issues layer-1 gate
    brackets/chain for step s+1 before layer-2 of step s, so the PE only
    ever waits on chains issued a full iteration earlier.  Layer-2 input
    gates wi1.T @ h1_new accumulate DIRECTLY into the layer-2 gate PSUM
    brackets (no separate gi2 phase, no DRAM roundtrip).
  - PSUM accumulation honors the 2KB zero-region rule: one start/stop
    bracket per bank (four single-bank pools per layer-half); the 12-16
    per-chunk results of a bank are disjoint 32-column slices accumulated
    inside a single bracket.  Phase A borrows the same rings so bank reuse
    across the phase boundary is WAR-ordered.
  - Gate biases ride on the PE as rank-1 (bias x ones) matmuls; layer-1
    input-gate r/z terms join via identity-matmul accumulation of bf16 gi1.
  - State is kept in bf16 (the update chain's final DVE add writes it
    directly); fp32 PSUM accumulation everywhere keeps the rel err ~6e-3
    against the fp32 reference (tolerance 2e-2).
"""

import numpy as np

try:
    import concourse.bass as bass  # noqa: F401
except ImportError:  # pragma: no cover - fallback for bare environments
    import sys
    sys.path.insert(0, "/opt/trn_rl_repo")
    import concourse.bass as bass  # noqa: F401

import ml_dtypes
import concourse.mybir as mybir
import concourse.tile as tile
from concourse import bacc
from concourse.bass_utils import run_bass_kernel_spmd
from concourse.masks import make_identity

F32 = mybir.dt.float32
BF16 = mybir.dt.bfloat16
AF = mybir.ActivationFunctionType
BFNP = ml_dtypes.bfloat16

H = 1024        # hidden size
G3 = 3 * H      # gate width
KC = H // 128   # K chunks (8)
MC = G3 // 128  # gate-col chunks (24)
T = 30          # sequences
R = 32          # padded runs per core (30 real + 2 pad)
RR = 30         # real runs (pad lanes skipped as matmul moving rows)
NSEG = 8
SUB = 4
NCORES = 8

# brow layout offsets (bf16 row vector of per-gate-col biases)
OFF_BHN1 = 0          # bh0 n-part      (H)
OFF_B2RZ = H          # (bi1+bh1) r/z   (2H)
OFF_BHN2 = 3 * H      # bh1 n-part      (H)
OFF_B2N = 4 * H       # bi1 n-part      (H)


def _rz_off(m):
    """Free offset of gate-chunk m (0..15 = r0-7, z0-7) inside the rz bank."""
    return m * 32 if m < 8 else 256 + (m - 8) * 32


def build_nc(steps=64):
    """Build the per-core Bass module (same program on all 8 cores)."""
    MT = steps * R            # gi1 row count
    nc = bacc.Bacc()

    xtr = nc.declare_dram_parameter("xtr", [128, KC, MT], BF16, isOutput=False)
    wi0s = nc.declare_dram_parameter("wi0s", [128, KC, G3], BF16, isOutput=False)
    wh0s = nc.declare_dram_parameter("wh0s", [128, KC, G3], BF16, isOutput=False)
    wi1s = nc.declare_dram_parameter("wi1s", [128, KC, G3], BF16, isOutput=False)
    wh1s = nc.declare_dram_parameter("wh1s", [128, KC, G3], BF16, isOutput=False)
    bias1c = nc.declare_dram_parameter("bias1c", [128, MC], F32, isOutput=False)
    brow = nc.declare_dram_parameter("brow", [1, 5 * H], BF16, isOutput=False)
    h1t0 = nc.declare_dram_parameter("h1t0", [128, KC, RR], BF16, isOutput=False)
    h2t0 = nc.declare_dram_parameter("h2t0", [128, KC, RR], BF16, isOutput=False)
    out = nc.declare_dram_parameter("out", [steps, 128, KC, RR], BF16,
                                    isOutput=True)

    gi1b = nc.dram_tensor("gi1b", [128, MC, steps * RR], BF16)

    with tile.TileContext(nc) as tc:
        with (
            tc.tile_pool(name="wp", bufs=3 * KC) as wp,
            tc.tile_pool(name="const", bufs=1) as const,
        ):
            # --- constants ---
            bias1c_t = const.tile([128, MC], F32)
            nc.sync.dma_start(out=bias1c_t, in_=bias1c[:])
            brow_t = const.tile([1, 5 * H], BF16)
            nc.sync.dma_start(out=brow_t, in_=brow[:])
            ident_f = const.tile([128, 128], F32)
            make_identity(nc, ident_f)
            ident = const.tile([128, 128], BF16)
            nc.vector.tensor_copy(ident, ident_f)
            ones = const.tile([1, R], BF16)
            nc.vector.memset(ones, 1.0)

            def load_weight(param, label, engines=None):
                tiles = []
                for k in range(KC):
                    wt = wp.tile([128, G3], BF16, tag="w", name=f"w_{label}_{k}")
                    eng = engines[k % len(engines)] if engines else nc.sync
                    eng.dma_start(out=wt, in_=param[:, k, :])
                    tiles.append(wt)
                return tiles

            # wi0 via the scalar queue so xtr streams on sync in parallel;
            # wh0/wh1 prefetch during phase A; wi1 recycles wi0's slots after
            wA = load_weight(wi0s, "i0", engines=[nc.scalar])

            # ---------------- shared PSUM rings (phase A + loop) -----------
            # 8 banks as four [128,512] single-bank rings; phase A borrows
            # the loop's rings so bank reuse across the phase boundary is
            # ordered by pool-slot WAR dependencies.
            with (
                tc.tile_pool(name="psB0", bufs=2, space="PSUM") as psB0p,
                tc.tile_pool(name="psB1", bufs=2, space="PSUM") as psB1p,
                tc.tile_pool(name="psD0", bufs=2, space="PSUM") as psD0p,
                tc.tile_pool(name="psD1", bufs=2, space="PSUM") as psD1p,
            ):
              ps_pools = [psB0p, psB1p, psD0p, psD1p]
              ps_tags = ["psb0", "psb1", "psd0", "psd1"]

              # ---------------- Phase A: gi1 (transposed dense) ------------
              NRB = MT // 512   # 512-row blocks
              with (
                tc.tile_pool(name="evA", bufs=6) as evA,
                tc.tile_pool(name="xp", bufs=KC) as xp,
              ):
                xts = []
                for k in range(KC):
                    xk = xp.tile([128, MT], BF16, tag="xt", name=f"xt_{k}")
                    nc.sync.dma_start(out=xk, in_=xtr[:, k, :])
                    xts.append(xk)
                # two m-chunks per block: 8 concurrent PSUM groups, k-outer
                # so consumption tracks the chunked k-ordered loads
                xvs = [x.rearrange("p (s t) -> p s t", t=R) for x in xts]
                SB = 16 * RR   # real rows per block (16 steps x 30 runs)
                for mb in range(0, MC, 2):
                    pss = [ps_pools[j % 4].tile(
                        [128, SB], F32, tag=ps_tags[j % 4],
                        name=f"psA_{mb}_{j}") for j in range(2 * NRB)]
                    for k in range(KC):
                        for dm in range(2):
                            m = mb + dm
                            for rb in range(NRB):
                                nc.tensor.matmul(
                                    pss[dm * NRB + rb],
                                    wA[k][:, m * 128:(m + 1) * 128],
                                    xvs[k][:, rb * 16:(rb + 1) * 16, 0:RR],
                                    start=(k == 0), stop=(k == KC - 1))
                    for dm in range(2):
                        m = mb + dm
                        for rb in range(NRB):
                            ev = evA.tile([128, SB], BF16, tag="ev",
                                          name=f"ev_{m}_{rb}")
                            nc.scalar.activation(ev, pss[dm * NRB + rb],
                                                 AF.Identity,
                                                 bias=bias1c_t[:, m:m + 1])
                            nc.sync.dma_start(
                                out=gi1b[:, m, rb * SB:(rb + 1) * SB], in_=ev)

              # remaining weights; sync queue stays free for per-step DMAs
              wB = load_weight(wh0s, "h0", engines=[nc.scalar])
              wC = load_weight(wi1s, "i1", engines=[nc.scalar])
              wD = load_weight(wh1s, "h1", engines=[nc.scalar])

              # ---------------- Fused recurrence loop -----------------------
              # PSUM bank layout, one 2KB bank-bracket per (layer, H-half):
              # (four independent [128,512] single-bank rings, 8 banks total)
              #   psB bank h: [r-h (4x32) | z-h | n-h | unused]
              #   psD bank h: [r-h | z-h | ngh-h | ngi-h]
              # Half h covers H-cols [512h, 512h+512) = state chunks 4h..4h+3,
              # i.e. gate m-chunks r: 4h+c, z: 8+4h+c, n: 16+4h+c (c in 0..3).
              # The loop is software-pipelined: iteration s issues the layer-1
              # bracket+chain for step s+1 BEFORE layer-2 of step s, so the PE
              # never waits on a chain issued in the same iteration.
              with (
                  tc.tile_pool(name="htb1p", bufs=3) as htb1p,
                  tc.tile_pool(name="htb2p", bufs=3) as htb2p,
                  tc.tile_pool(name="gip", bufs=6) as gip,
                  tc.tile_pool(name="gp", bufs=7) as gp,
              ):
                  htb1 = htb1p.tile([128, KC, RR], BF16, tag="htb1", name="htb1_init")
                  nc.sync.dma_start(out=htb1, in_=h1t0[:])
                  htb2 = htb2p.tile([128, KC, RR], BF16, tag="htb2", name="htb2_init")
                  nc.sync.dma_start(out=htb2, in_=h2t0[:])

                  def bank3(ps_h):
                      # [128, 512] bank -> [128, 16, 32] (chunk, slot) view
                      return ps_h.rearrange("p (c r) -> p c r", r=32)

                  def rank1(dst, boff, m, stop):
                      nc.tensor.matmul(
                          dst, brow_t[:, boff + m * 128:boff + (m + 1) * 128],
                          ones[:, 0:RR], start=False, stop=stop)

                  def gh_group(ps, h, q, c, w, htb, start, kr=range(KC)):
                      """Recurrent matmuls into quarter q, slot c of bank h."""
                      m = (0, 8, 16, 16)[q] + 4 * h + c
                      dst = ps[h][:, q * 128 + c * 32:q * 128 + c * 32 + RR]
                      for k in kr:
                          nc.tensor.matmul(
                              dst, w[k][:, m * 128:(m + 1) * 128],
                              htb[:, k, :],
                              start=(start and k == kr[0]), stop=False)

                  def update_half(nm, s, h, ps, gin_ap, htb_cur, htb_new):
                      a, b = 4 * h, 4 * h + 4
                      t = lambda lbl, sh: gp.tile([128, sh, RR], F32, tag=lbl,
                                                  name=f"{lbl}{nm}_{s}_{h}")
                      rz = t("rz", 8)
                      nc.scalar.activation(rz, bank3(ps[h])[:, 0:8, 0:RR],
                                           AF.Sigmoid)
                      t1 = t("t1", 4)
                      nc.vector.tensor_mul(t1, rz[:, 0:4, :],
                                           bank3(ps[h])[:, 8:12, 0:RR])
                      npre = t("np", 4)
                      nc.vector.tensor_add(npre, t1, gin_ap)
                      nn = t("nn", 4)
                      nc.scalar.activation(nn, npre, AF.Tanh)
                      dd = t("dd", 4)
                      nc.vector.tensor_sub(dd, htb_cur[:, a:b, :], nn)
                      t2 = t("t2", 4)
                      nc.vector.tensor_mul(t2, rz[:, 4:8, :], dd)
                      nc.vector.tensor_add(htb_new[:, a:b, :], nn, t2)

                  def bracket_B(s, psb, gi, htb_cur, h):
                      # pass 1: state chunks 0-3 for every group (tolerates
                      # the previous chain's h1 half still being in flight)
                      for q in range(3):
                          for c in range(4):
                              gh_group(psb, h, q, c, wB, htb_cur,
                                       start=(q == 0 and c == 0),
                                       kr=range(0, 4))
                      # pass 2: chunks 4-7 + gi/bias closers
                      for q in range(3):
                          for c in range(4):
                              gh_group(psb, h, q, c, wB, htb_cur,
                                       start=False, kr=range(4, KC))
                              m = (0, 8, 16)[q] + 4 * h + c
                              dst = psb[h][:, q * 128 + c * 32:
                                           q * 128 + c * 32 + RR]
                              if q < 2:
                                  nc.tensor.matmul(
                                      dst, ident, gi[:, m, :], start=False,
                                      stop=(q == 2 and c == 3))
                              else:
                                  rank1(dst, OFF_BHN1, 4 * h + c,
                                        stop=(q == 2 and c == 3))

                  # software-pipelined steady state; gi prefetched 2 steps out
                  def gi_fetch(j):
                      g = gip.tile([128, MC, RR], BF16, tag="gi", name=f"gi_{j}")
                      nc.sync.dma_start(out=g,
                                        in_=gi1b[:, :, j * RR:(j + 1) * RR])
                      return g

                  gi_tiles = {j: gi_fetch(j) for j in range(min(3, steps))}
                  for s in range(-1, steps):
                      if s + 4 < steps:
                          gi_tiles[s + 4] = gi_fetch(s + 4)
                      # --- issue layer-1 of step s+1 (h0; h1 goes between
                      # the layer-2 gh2 half-blocks) ---
                      psb_n = None
                      if s + 1 < steps:
                          gi_n = gi_tiles.pop(s + 1)
                          psb_n = [
                              psB0p.tile([128, 512], F32, tag="psb0",
                                         name=f"psb0_{s + 1}"),
                              psB1p.tile([128, 512], F32, tag="psb1",
                                         name=f"psb1_{s + 1}")]
                          bracket_B(s + 1, psb_n, gi_n, htb1, 0)
                          htb1n = htb1p.tile([128, KC, RR], BF16, tag="htb1",
                                             name=f"htb1_{s + 1}")
                          if s < 0:
                              bracket_B(s + 1, psb_n, gi_n, htb1, 1)
                              update_half("B", s + 1, 0, psb_n,
                                          gi_n[:, 16:20, :], htb1, htb1n)
                              update_half("B", s + 1, 1, psb_n,
                                          gi_n[:, 20:24, :], htb1, htb1n)
                      else:
                          htb1n = None

                      if s >= 0:
                          # --- layer-2 of step s (htb1 = state after step s) ---
                          psd = [
                              psD0p.tile([128, 512], F32, tag="psd0",
                                         name=f"psd0_{s}"),
                              psD1p.tile([128, 512], F32, tag="psd1",
                                         name=f"psd1_{s}")]

                          def d_part1(h):
                              for q in range(3):
                                  for c in range(4):
                                      gh_group(psd, h, q, c, wD, htb2,
                                               start=(q == 0 and c == 0))
                                      if q == 2:
                                          dst = psd[h][:, 256 + c * 32:
                                                       256 + c * 32 + RR]
                                          rank1(dst, OFF_BHN2, 4 * h + c,
                                                stop=False)
                          if psb_n is not None:
                              bracket_B(s + 1, psb_n, gi_n, htb1, 1)
                          d_part1(0)
                          d_part1(1)
                          if psb_n is not None:
                              update_half("B", s + 1, 0, psb_n,
                                          gi_n[:, 16:20, :], htb1, htb1n)
                              update_half("B", s + 1, 1, psb_n,
                                          gi_n[:, 20:24, :], htb1, htb1n)
                          for h in (0, 1):
                              for q in range(4):
                                  qq = (0, 8, 0, 16)[q]
                                  for c in range(4):
                                      if q == 2:
                                          continue
                                      m = qq + 4 * h + c
                                      dst = psd[h][:, q * 128 + c * 32:
                                                   q * 128 + c * 32 + RR]
                                      for k in range(KC):
                                          nc.tensor.matmul(
                                              dst,
                                              wC[k][:, m * 128:(m + 1) * 128],
                                              htb1[:, k, 0:RR],
                                              start=False, stop=False)
                              for c in range(4):
                                  rank1(psd[h][:, c * 32:c * 32 + RR],
                                        OFF_B2RZ, 4 * h + c, stop=False)
                                  rank1(psd[h][:, 128 + c * 32:128 + c * 32 + RR],
                                        OFF_B2RZ, 8 + 4 * h + c, stop=False)
                                  rank1(psd[h][:, 384 + c * 32:384 + c * 32 + RR],
                                        OFF_B2N, 4 * h + c, stop=(c == 3))

                          htb2n = htb2p.tile([128, KC, RR], BF16, tag="htb2",
                                             name=f"htb2_{s}")
                          update_half("D", s, 0, psd,
                                      bank3(psd[0])[:, 12:16, 0:RR],
                                      htb2, htb2n)
                          update_half("D", s, 1, psd,
                                      bank3(psd[1])[:, 12:16, 0:RR],
                                      htb2, htb2n)
                          nc.sync.dma_start(out=out[s], in_=htb2n)
                          htb2 = htb2n

                      if htb1n is not None:
                          htb1 = htb1n

    nc.finalize()
    return nc


def ode_traj(w1, b1, w2, b2, w3, b3):
    """RK4 trajectory of the ODE, mirroring the reference exactly (fp32)."""
    w1t = w1.T.astype(np.float32)
    w2t = w2.T.astype(np.float32)
    w3t = w3.T.astype(np.float32)

    def f(h):
        a = np.tanh(h @ w1t + b1)
        a = np.tanh(a @ w2t + b2)
        return a @ w3t + b3

    dt = np.float32((1.0 / NSEG) / SUB)
    h = np.zeros((2, H), np.float32)
    traj = []
    for _ in range(NSEG):
        for _ in range(SUB):
            k1 = f(h)
            k2 = f(h + np.float32(0.5) * dt * k1)
            k3 = f(h + np.float32(0.5) * dt * k2)
            k4 = f(h + dt * k3)
            h = h + (dt / np.float32(6.0)) * (k1 + np.float32(2.0) * k2
                                              + np.float32(2.0) * k3 + k4)
        traj.append(h.copy())
    return np.stack(traj)  # (NSEG, 2, H)


def _wstat(w):
    """[G3, H] weight -> stationary layout [128, KC, G3] (bf16)."""
    return np.ascontiguousarray(
        w.T.reshape(KC, 128, G3).transpose(1, 0, 2)).astype(BFNP)


def make_in_maps(x, w1, b1, w2, b2, w3, b3, wi0, wh0, bi0, bh0,
                 wi1, wh1, bi1, bh1, steps=64, cores=NCORES):
    traj = ode_traj(w1, b1, w2, b2, w3, b3)
    MT = steps * R

    xp = np.zeros((steps, R, H), np.float32)
    xp[:, :T, :] = x[:steps, :, :]
    xtr = np.ascontiguousarray(
        xp.reshape(MT, KC, 128).transpose(2, 1, 0)).astype(BFNP)

    bias1full = np.concatenate([bi0[:2 * H] + bh0[:2 * H], bi0[2 * H:]])
    brow = np.concatenate([bh0[2 * H:],                      # OFF_BHN1
                           bi1[:2 * H] + bh1[:2 * H],        # OFF_B2RZ
                           bh1[2 * H:],                      # OFF_BHN2
                           bi1[2 * H:]])                     # OFF_B2N

    shared = {
        "xtr": xtr,
        "wi0s": _wstat(wi0), "wh0s": _wstat(wh0),
        "wi1s": _wstat(wi1), "wh1s": _wstat(wh1),
        "bias1c": np.ascontiguousarray(
            bias1full.reshape(MC, 128).T).astype(np.float32),
        "brow": brow[None].astype(BFNP),
    }
    in_maps = []
    for i in range(cores):
        m = dict(shared)
        for nm, hv in (("h1t0", traj[i, 0]), ("h2t0", traj[i, 1])):
            ht = np.broadcast_to(
                hv.reshape(KC, 128).T[:, :, None], (128, KC, RR))
            m[nm] = np.ascontiguousarray(ht).astype(BFNP)
        in_maps.append(m)
    return in_maps


_NC_CACHE = {}


def _get_nc(steps):
    if steps not in _NC_CACHE:
        _NC_CACHE[steps] = build_nc(steps)
    return _NC_CACHE[steps]


def run_cores(inputs, steps=64, cores=NCORES, **run_kwargs):
    in_maps = make_in_maps(steps=steps, cores=cores, **inputs)
    nc = _get_nc(steps)
    return run_bass_kernel_spmd(nc, in_maps, core_ids=list(range(cores)),
                                **run_kwargs)


def kernel(x, w1, b1, w2, b2, w3, b3, wi0, wh0, bi0, bh0,
           wi1, wh1, bi1, bh1):
    args = dict(x=x, w1=w1, b1=b1, w2=w2, b2=b2, w3=w3, b3=b3,
                wi0=wi0, wh0=wh0, bi0=bi0, bh0=bh0,
                wi1=wi1, wh1=wh1, bi1=bi1, bh1=bh1)
    args = {k: np.asarray(v, np.float32) for k, v in args.items()}
    res = run_cores(args, steps=64, cores=NCORES)
    B = 64
    full = np.empty((B, T * NCORES, H), np.float32)
    for i in range(NCORES):
        r = np.asarray(res.results[i]["out"], np.float32)  # (64,128,KC,RR)
        r = np.transpose(r, (0, 3, 2, 1)).reshape(B, RR, H)
        full[:, i::NCORES, :] = r
    return full



# revision 24
# speedup vs baseline: 3.2062x; 1.0156x over previous
"""ODE-RNN Trainium2 kernel (transposed-state, weights-stationary bf16).

Problem: out[b, t*8+i, :] = 2-layer GRU (H=1024) run over the batch dim
(64 steps) of sequence t (30 sequences), with initial hiddens taken from an
RK4-integrated ODE trajectory (8 grid points, shared across all runs).

Strategy (8 NeuronCores, pure data-parallel, no collectives):
  - The ODE trajectory (tiny, latency-serial) is computed on the host in
    fp32, exactly mirroring the reference math.
  - Core i handles the 30 GRU runs with init traj[i] (data-parallel over
    the 240 independent (t,i) runs; weights replicated per core).
  - Everything on-device lives in a TRANSPOSED layout: states/gates are
    [128 H-cols (partitions), chunk, run] so elementwise work uses all 128
    partitions, and recurrence matmuls make the small per-step state the
    MOVING operand (30 bf16 rows/matmul at 1 cycle/row) with the weights
    stationary - ~4x fewer PE row-cycles than streaming the weights.
  - Phase A: gi1 = x @ wi0.T + bias as a dense transposed matmul -> DRAM
    (bias folded into the PSUM->bf16 eviction on the Activation engine).
  - Software-pipelined fused step loop: iteration s issues layer-1 gate
    brackets/chain for step s+1 before layer-2 of step s, so the PE only
    ever waits on chains issued a full iteration earlier.  Layer-2 input
    gates wi1.T @ h1_new accumulate DIRECTLY into the layer-2 gate PSUM
    brackets (no separate gi2 phase, no DRAM roundtrip).
  - PSUM accumulation honors the 2KB zero-region rule: one start/stop
    bracket per bank (four single-bank pools per layer-half); the 12-16
    per-chunk results of a bank are disjoint 32-column slices accumulated
    inside a single bracket.  Phase A borrows the same rings so bank reuse
    across the phase boundary is WAR-ordered.
  - Gate biases ride on the PE as rank-1 (bias x ones) matmuls; layer-1
    input-gate r/z terms join via identity-matmul accumulation of bf16 gi1.
  - State is kept in bf16 (the update chain's final DVE add writes it
    directly); fp32 PSUM accumulation everywhere keeps the rel err ~6e-3
    against the fp32 reference (tolerance 2e-2).
"""

import numpy as np

try:
    import concourse.bass as bass  # noqa: F401
except ImportError:  # pragma: no cover - fallback for bare environments
    import sys
    sys.path.insert(0, "/opt/trn_rl_repo")
    import concourse.bass as bass  # noqa: F401

import ml_dtypes
import concourse.mybir as mybir
import concourse.tile as tile
from concourse import bacc
from concourse.bass_utils import run_bass_kernel_spmd
from concourse.masks import make_identity

F32 = mybir.dt.float32
BF16 = mybir.dt.bfloat16
AF = mybir.ActivationFunctionType
BFNP = ml_dtypes.bfloat16

H = 1024        # hidden size
G3 = 3 * H      # gate width
KC = H // 128   # K chunks (8)
MC = G3 // 128  # gate-col chunks (24)
T = 30          # sequences
R = 32          # padded runs per core (30 real + 2 pad)
RR = 30         # real runs (pad lanes skipped as matmul moving rows)
NSEG = 8
SUB = 4
NCORES = 8

# brow layout offsets (bf16 row vector of per-gate-col biases)
OFF_BHN1 = 0          # bh0 n-part      (H)
OFF_B2RZ = H          # (bi1+bh1) r/z   (2H)
OFF_BHN2 = 3 * H      # bh1 n-part      (H)
OFF_B2N = 4 * H       # bi1 n-part      (H)


def _rz_off(m):
    """Free offset of gate-chunk m (0..15 = r0-7, z0-7) inside the rz bank."""
    return m * 32 if m < 8 else 256 + (m - 8) * 32


def build_nc(steps=64):
    """Build the per-core Bass module (same program on all 8 cores)."""
    MT = steps * R            # gi1 row count
    nc = bacc.Bacc()

    xtr = nc.declare_dram_parameter("xtr", [128, KC, MT], BF16, isOutput=False)
    wi0s = nc.declare_dram_parameter("wi0s", [128, KC, G3], BF16, isOutput=False)
    wh0s = nc.declare_dram_parameter("wh0s", [128, KC, G3], BF16, isOutput=False)
    wi1s = nc.declare_dram_parameter("wi1s", [128, KC, G3], BF16, isOutput=False)
    wh1s = nc.declare_dram_parameter("wh1s", [128, KC, G3], BF16, isOutput=False)
    bias1c = nc.declare_dram_parameter("bias1c", [128, MC], F32, isOutput=False)
    brow = nc.declare_dram_parameter("brow", [1, 5 * H], BF16, isOutput=False)
    h1t0 = nc.declare_dram_parameter("h1t0", [128, KC, RR], BF16, isOutput=False)
    h2t0 = nc.declare_dram_parameter("h2t0", [128, KC, RR], BF16, isOutput=False)
    out = nc.declare_dram_parameter("out", [steps, 128, KC, RR], BF16,
                                    isOutput=True)

    gi1b = nc.dram_tensor("gi1b", [128, MC, steps * RR], BF16)

    with tile.TileContext(nc) as tc:
        with (
            tc.tile_pool(name="wp", bufs=3 * KC) as wp,
            tc.tile_pool(name="const", bufs=1) as const,
        ):
            # --- constants ---
            bias1c_t = const.tile([128, MC], F32)
            nc.sync.dma_start(out=bias1c_t, in_=bias1c[:])
            brow_t = const.tile([1, 5 * H], BF16)
            nc.sync.dma_start(out=brow_t, in_=brow[:])
            ident_f = const.tile([128, 128], F32)
            make_identity(nc, ident_f)
            ident = const.tile([128, 128], BF16)
            nc.vector.tensor_copy(ident, ident_f)
            ones = const.tile([1, R], BF16)
            nc.vector.memset(ones, 1.0)

            def load_weight(param, label, engines=None):
                tiles = []
                for k in range(KC):
                    wt = wp.tile([128, G3], BF16, tag="w", name=f"w_{label}_{k}")
                    eng = engines[k % len(engines)] if engines else nc.sync
                    eng.dma_start(out=wt, in_=param[:, k, :])
                    tiles.append(wt)
                return tiles

            # wi0 via the scalar queue so xtr streams on sync in parallel;
            # wh0/wh1 prefetch during phase A; wi1 recycles wi0's slots after
            wA = load_weight(wi0s, "i0", engines=[nc.scalar])

            # ---------------- shared PSUM rings (phase A + loop) -----------
            # 8 banks as four [128,512] single-bank rings; phase A borrows
            # the loop's rings so bank reuse across the phase boundary is
            # ordered by pool-slot WAR dependencies.
            with (
                tc.tile_pool(name="psB0", bufs=2, space="PSUM") as psB0p,
                tc.tile_pool(name="psB1", bufs=2, space="PSUM") as psB1p,
                tc.tile_pool(name="psD0", bufs=2, space="PSUM") as psD0p,
                tc.tile_pool(name="psD1", bufs=2, space="PSUM") as psD1p,
            ):
              ps_pools = [psB0p, psB1p, psD0p, psD1p]
              ps_tags = ["psb0", "psb1", "psd0", "psd1"]

              # ---------------- Phase A: gi1 (transposed dense) ------------
              NRB = MT // 512   # 512-row blocks
              with (
                tc.tile_pool(name="evA", bufs=6) as evA,
                tc.tile_pool(name="xp", bufs=KC) as xp,
              ):
                xts = []
                for k in range(KC):
                    xk = xp.tile([128, MT], BF16, tag="xt", name=f"xt_{k}")
                    nc.sync.dma_start(out=xk, in_=xtr[:, k, :])
                    xts.append(xk)
                # two m-chunks per block: 8 concurrent PSUM groups, k-outer
                # so consumption tracks the chunked k-ordered loads
                xvs = [x.rearrange("p (s t) -> p s t", t=R) for x in xts]
                SB = 16 * RR   # real rows per block (16 steps x 30 runs)
                for mb in range(0, MC, 2):
                    pss = [ps_pools[j % 4].tile(
                        [128, SB], F32, tag=ps_tags[j % 4],
                        name=f"psA_{mb}_{j}") for j in range(2 * NRB)]
                    for k in range(KC):
                        for dm in range(2):
                            m = mb + dm
                            for rb in range(NRB):
                                nc.tensor.matmul(
                                    pss[dm * NRB + rb],
                                    wA[k][:, m * 128:(m + 1) * 128],
                                    xvs[k][:, rb * 16:(rb + 1) * 16, 0:RR],
                                    start=(k == 0), stop=(k == KC - 1))
                    for dm in range(2):
                        m = mb + dm
                        for rb in range(NRB):
                            ev = evA.tile([128, SB], BF16, tag="ev",
                                          name=f"ev_{m}_{rb}")
                            nc.scalar.activation(ev, pss[dm * NRB + rb],
                                                 AF.Identity,
                                                 bias=bias1c_t[:, m:m + 1])
                            nc.sync.dma_start(
                                out=gi1b[:, m, rb * SB:(rb + 1) * SB], in_=ev)

              # remaining weights; sync queue stays free for per-step DMAs
              wB = load_weight(wh0s, "h0", engines=[nc.scalar])
              wC = load_weight(wi1s, "i1", engines=[nc.scalar])
              wD = load_weight(wh1s, "h1", engines=[nc.scalar])

              # ---------------- Fused recurrence loop -----------------------
              # PSUM bank layout, one 2KB bank-bracket per (layer, H-half):
              # (four independent [128,512] single-bank rings, 8 banks total)
              #   psB bank h: [r-h (4x32) | z-h | n-h | unused]
              #   psD bank h: [r-h | z-h | ngh-h | ngi-h]
              # Half h covers H-cols [512h, 512h+512) = state chunks 4h..4h+3,
              # i.e. gate m-chunks r: 4h+c, z: 8+4h+c, n: 16+4h+c (c in 0..3).
              # The loop is software-pipelined: iteration s issues the layer-1
              # bracket+chain for step s+1 BEFORE layer-2 of step s, so the PE
              # never waits on a chain issued in the same iteration.
              with (
                  tc.tile_pool(name="htb1p", bufs=3) as htb1p,
                  tc.tile_pool(name="htb2p", bufs=3) as htb2p,
                  tc.tile_pool(name="gip", bufs=6) as gip,
                  tc.tile_pool(name="gp", bufs=7) as gp,
              ):
                  htb1 = htb1p.tile([128, KC, RR], BF16, tag="htb1", name="htb1_init")
                  nc.sync.dma_start(out=htb1, in_=h1t0[:])
                  htb2 = htb2p.tile([128, KC, RR], BF16, tag="htb2", name="htb2_init")
                  nc.sync.dma_start(out=htb2, in_=h2t0[:])

                  def bank3(ps_h):
                      # [128, 512] bank -> [128, 16, 32] (chunk, slot) view
                      return ps_h.rearrange("p (c r) -> p c r", r=32)

                  def rank1(dst, boff, m, stop):
                      nc.tensor.matmul(
                          dst, brow_t[:, boff + m * 128:boff + (m + 1) * 128],
                          ones[:, 0:RR], start=False, stop=stop)

                  def gh_group(ps, h, q, c, w, htb, start, kr=range(KC)):
                      """Recurrent matmuls into quarter q, slot c of bank h."""
                      m = (0, 8, 16, 16)[q] + 4 * h + c
                      dst = ps[h][:, q * 128 + c * 32:q * 128 + c * 32 + RR]
                      for k in kr:
                          nc.tensor.matmul(
                              dst, w[k][:, m * 128:(m + 1) * 128],
                              htb[:, k, :],
                              start=(start and k == kr[0]), stop=False)

                  def update_half(nm, s, h, ps, gin_ap, htb_cur, htb_new):
                      a, b = 4 * h, 4 * h + 4
                      t = lambda lbl, sh: gp.tile([128, sh, RR], F32, tag=lbl,
                                                  name=f"{lbl}{nm}_{s}_{h}")
                      rz = t("rz", 8)
                      nc.scalar.activation(rz, bank3(ps[h])[:, 0:8, 0:RR],
                                           AF.Sigmoid)
                      t1 = t("t1", 4)
                      nc.vector.tensor_mul(t1, rz[:, 0:4, :],
                                           bank3(ps[h])[:, 8:12, 0:RR])
                      npre = t("np", 4)
                      nc.vector.tensor_add(npre, t1, gin_ap)
                      # h' = (1-z)*n + z*h: z*h and (1-z) overlap the tanh,
                      # leaving only two DVE ops after nn on the chain
                      e1 = t("e1", 4)
                      nc.vector.tensor_mul(e1, rz[:, 4:8, :], htb_cur[:, a:b, :])
                      u = t("u", 4)
                      nc.vector.tensor_scalar(u, rz[:, 4:8, :], -1.0, 1.0,
                                              mybir.AluOpType.mult,
                                              mybir.AluOpType.add)
                      nn = t("nn", 4)
                      nc.scalar.activation(nn, npre, AF.Tanh)
                      w = t("w", 4)
                      nc.vector.tensor_mul(w, u, nn)
                      nc.vector.tensor_add(htb_new[:, a:b, :], w, e1)

                  def bracket_B(s, psb, gi, htb_cur, h):
                      # pass 1: state chunks 0-3 for every group (tolerates
                      # the previous chain's h1 half still being in flight)
                      for q in range(3):
                          for c in range(4):
                              gh_group(psb, h, q, c, wB, htb_cur,
                                       start=(q == 0 and c == 0),
                                       kr=range(0, 4))
                      # pass 2: chunks 4-7 + gi/bias closers
                      for q in range(3):
                          for c in range(4):
                              gh_group(psb, h, q, c, wB, htb_cur,
                                       start=False, kr=range(4, KC))
                              m = (0, 8, 16)[q] + 4 * h + c
                              dst = psb[h][:, q * 128 + c * 32:
                                           q * 128 + c * 32 + RR]
                              if q < 2:
                                  nc.tensor.matmul(
                                      dst, ident, gi[:, m, :], start=False,
                                      stop=(q == 2 and c == 3))
                              else:
                                  rank1(dst, OFF_BHN1, 4 * h + c,
                                        stop=(q == 2 and c == 3))

                  # software-pipelined steady state; gi prefetched 2 steps out
                  def gi_fetch(j):
                      g = gip.tile([128, MC, RR], BF16, tag="gi", name=f"gi_{j}")
                      nc.sync.dma_start(out=g,
                                        in_=gi1b[:, :, j * RR:(j + 1) * RR])
                      return g

                  gi_tiles = {j: gi_fetch(j) for j in range(min(3, steps))}
                  for s in range(-1, steps):
                      if s + 4 < steps:
                          gi_tiles[s + 4] = gi_fetch(s + 4)
                      # --- issue layer-1 of step s+1 (h0; h1 goes between
                      # the layer-2 gh2 half-blocks) ---
                      psb_n = None
                      if s + 1 < steps:
                          gi_n = gi_tiles.pop(s + 1)
                          psb_n = [
                              psB0p.tile([128, 512], F32, tag="psb0",
                                         name=f"psb0_{s + 1}"),
                              psB1p.tile([128, 512], F32, tag="psb1",
                                         name=f"psb1_{s + 1}")]
                          bracket_B(s + 1, psb_n, gi_n, htb1, 0)
                          htb1n = htb1p.tile([128, KC, RR], BF16, tag="htb1",
                                             name=f"htb1_{s + 1}")
                          if s < 0:
                              bracket_B(s + 1, psb_n, gi_n, htb1, 1)
                              update_half("B", s + 1, 0, psb_n,
                                          gi_n[:, 16:20, :], htb1, htb1n)
                              update_half("B", s + 1, 1, psb_n,
                                          gi_n[:, 20:24, :], htb1, htb1n)
                      else:
                          htb1n = None

                      if s >= 0:
                          # --- layer-2 of step s (htb1 = state after step s) ---
                          psd = [
                              psD0p.tile([128, 512], F32, tag="psd0",
                                         name=f"psd0_{s}"),
                              psD1p.tile([128, 512], F32, tag="psd1",
                                         name=f"psd1_{s}")]

                          def d_part1(h):
                              for q in range(3):
                                  for c in range(4):
                                      gh_group(psd, h, q, c, wD, htb2,
                                               start=(q == 0 and c == 0))
                                      if q == 2:
                                          dst = psd[h][:, 256 + c * 32:
                                                       256 + c * 32 + RR]
                                          rank1(dst, OFF_BHN2, 4 * h + c,
                                                stop=False)
                          if psb_n is not None:
                              bracket_B(s + 1, psb_n, gi_n, htb1, 1)
                          d_part1(0)
                          d_part1(1)
                          if psb_n is not None:
                              update_half("B", s + 1, 0, psb_n,
                                          gi_n[:, 16:20, :], htb1, htb1n)
                              update_half("B", s + 1, 1, psb_n,
                                          gi_n[:, 20:24, :], htb1, htb1n)
                          for h in (0, 1):
                              for q in range(4):
                                  qq = (0, 8, 0, 16)[q]
                                  for c in range(4):
                                      if q == 2:
                                          continue
                                      m = qq + 4 * h + c
                                      dst = psd[h][:, q * 128 + c * 32:
                                                   q * 128 + c * 32 + RR]
                                      for k in range(KC):
                                          nc.tensor.matmul(
                                              dst,
                                              wC[k][:, m * 128:(m + 1) * 128],
                                              htb1[:, k, 0:RR],
                                              start=False, stop=False)
                              for c in range(4):
                                  rank1(psd[h][:, c * 32:c * 32 + RR],
                                        OFF_B2RZ, 4 * h + c, stop=False)
                                  rank1(psd[h][:, 128 + c * 32:128 + c * 32 + RR],
                                        OFF_B2RZ, 8 + 4 * h + c, stop=False)
                                  rank1(psd[h][:, 384 + c * 32:384 + c * 32 + RR],
                                        OFF_B2N, 4 * h + c, stop=(c == 3))

                          htb2n = htb2p.tile([128, KC, RR], BF16, tag="htb2",
                                             name=f"htb2_{s}")
                          update_half("D", s, 0, psd,
                                      bank3(psd[0])[:, 12:16, 0:RR],
                                      htb2, htb2n)
                          update_half("D", s, 1, psd,
                                      bank3(psd[1])[:, 12:16, 0:RR],
                                      htb2, htb2n)
                          nc.sync.dma_start(out=out[s], in_=htb2n)
                          htb2 = htb2n

                      if htb1n is not None:
                          htb1 = htb1n

    nc.finalize()
    return nc


def ode_traj(w1, b1, w2, b2, w3, b3):
    """RK4 trajectory of the ODE, mirroring the reference exactly (fp32)."""
    w1t = w1.T.astype(np.float32)
    w2t = w2.T.astype(np.float32)
    w3t = w3.T.astype(np.float32)

    def f(h):
        a = np.tanh(h @ w1t + b1)
        a = np.tanh(a @ w2t + b2)
        return a @ w3t + b3

    dt = np.float32((1.0 / NSEG) / SUB)
    h = np.zeros((2, H), np.float32)
    traj = []
    for _ in range(NSEG):
        for _ in range(SUB):
            k1 = f(h)
            k2 = f(h + np.float32(0.5) * dt * k1)
            k3 = f(h + np.float32(0.5) * dt * k2)
            k4 = f(h + dt * k3)
            h = h + (dt / np.float32(6.0)) * (k1 + np.float32(2.0) * k2
                                              + np.float32(2.0) * k3 + k4)
        traj.append(h.copy())
    return np.stack(traj)  # (NSEG, 2, H)


def _wstat(w):
    """[G3, H] weight -> stationary layout [128, KC, G3] (bf16)."""
    return np.ascontiguousarray(
        w.T.reshape(KC, 128, G3).transpose(1, 0, 2)).astype(BFNP)


def make_in_maps(x, w1, b1, w2, b2, w3, b3, wi0, wh0, bi0, bh0,
                 wi1, wh1, bi1, bh1, steps=64, cores=NCORES):
    traj = ode_traj(w1, b1, w2, b2, w3, b3)
    MT = steps * R

    xp = np.zeros((steps, R, H), np.float32)
    xp[:, :T, :] = x[:steps, :, :]
    xtr = np.ascontiguousarray(
        xp.reshape(MT, KC, 128).transpose(2, 1, 0)).astype(BFNP)

    bias1full = np.concatenate([bi0[:2 * H] + bh0[:2 * H], bi0[2 * H:]])
    brow = np.concatenate([bh0[2 * H:],                      # OFF_BHN1
                           bi1[:2 * H] + bh1[:2 * H],        # OFF_B2RZ
                           bh1[2 * H:],                      # OFF_BHN2
                           bi1[2 * H:]])                     # OFF_B2N

    shared = {
        "xtr": xtr,
        "wi0s": _wstat(wi0), "wh0s": _wstat(wh0),
        "wi1s": _wstat(wi1), "wh1s": _wstat(wh1),
        "bias1c": np.ascontiguousarray(
            bias1full.reshape(MC, 128).T).astype(np.float32),
        "brow": brow[None].astype(BFNP),
    }
    in_maps = []
    for i in range(cores):
        m = dict(shared)
        for nm, hv in (("h1t0", traj[i, 0]), ("h2t0", traj[i, 1])):
            ht = np.broadcast_to(
                hv.reshape(KC, 128).T[:, :, None], (128, KC, RR))
            m[nm] = np.ascontiguousarray(ht).astype(BFNP)
        in_maps.append(m)
    return in_maps


_NC_CACHE = {}


def _get_nc(steps):
    if steps not in _NC_CACHE:
        _NC_CACHE[steps] = build_nc(steps)
    return _NC_CACHE[steps]


def run_cores(inputs, steps=64, cores=NCORES, **run_kwargs):
    in_maps = make_in_maps(steps=steps, cores=cores, **inputs)
    nc = _get_nc(steps)
    return run_bass_kernel_spmd(nc, in_maps, core_ids=list(range(cores)),
                                **run_kwargs)


def kernel(x, w1, b1, w2, b2, w3, b3, wi0, wh0, bi0, bh0,
           wi1, wh1, bi1, bh1):
    args = dict(x=x, w1=w1, b1=b1, w2=w2, b2=b2, w3=w3, b3=b3,
                wi0=wi0, wh0=wh0, bi0=bi0, bh0=bh0,
                wi1=wi1, wh1=wh1, bi1=bi1, bh1=bh1)
    args = {k: np.asarray(v, np.float32) for k, v in args.items()}
    res = run_cores(args, steps=64, cores=NCORES)
    B = 64
    full = np.empty((B, T * NCORES, H), np.float32)
    for i in range(NCORES):
        r = np.asarray(res.results[i]["out"], np.float32)  # (64,128,KC,RR)
        r = np.transpose(r, (0, 3, 2, 1)).reshape(B, RR, H)
        full[:, i::NCORES, :] = r
    return full

